# revision 5
# baseline (speedup 1.0000x reference)
"""Trainium2 Bass kernel for nn_MemTransformerLM (DPFP linear-attention block).

Full inputs in, full output out. Internally: head-shards across 8 NeuronCores
(2 heads/core), runs causal linear attention as a chunked prefix-sum (the
reference's sum-normalized kernelized attention factorizes: no SxS score
materialization), AllToAll re-shards heads->rows for the output projection,
and each core LayerNorms its row slice. Host concatenates the 8 row slices.

v2 structure (vs v1):
- per-head attention buffers so head-0's AllToAll stores don't falsely
  depend on head-1's copies (partition aliasing in dep tracking)
- attention emits u^T [d, i] directly (intra term va^T @ probT, state term
  KV^T @ qf) so no output transpose / scalar copy; normalization uses a
  K=1 PE broadcast of the reciprocal denominator row
- o-projection split per head: head-0's partial (x = hs + attn0 @ Wo0)
  runs while head-1's AllToAll is in flight, keeping the PE warm through
  the collective and leaving only head-1's partial + LayerNorm on the tail
- o-projection stacks rank pairs to use all 128 PE rows (K=128 not 64)
- phase-1/2 buffers freed before attention so phase-4 weights stay resident
- DPFP products split across Vector and GpSimd(Pool) engines
"""
import os
import sys
import types
from contextlib import ExitStack

for _p in ("/opt/trn_rl_repo",):
    if _p not in sys.path:
        sys.path.insert(0, _p)

import numpy as np
import ml_dtypes

import concourse.bass as bass
import concourse.mybir as mybir
import concourse.tile as tile
from concourse import bacc
from concourse.bass_utils import run_bass_kernel_spmd

BF16 = ml_dtypes.bfloat16
F32 = np.float32

SEQ, BATCH, D = 1536, 2, 1024
NH, DH, NR = 16, 64, 3
SCALE = 1.0 / float(np.sqrt(DH))
S_FOLD = float(np.sqrt(SCALE))           # folded into Wq rows (squared by DPFP products)
EPS_D, EPS_LN = 1e-5, 1e-5
N_CORES = 8
HPC = NH // N_CORES                      # heads per core (2)
ROWS = SEQ * BATCH                       # 3072 batch-major rows
RPC = ROWS // N_CORES                    # 384 output rows per core
NCHUNK = ROWS // 128                     # 24 chunks of 128 rows
NCB = NCHUNK // BATCH                    # 12 chunks per batch
FEAT = 2 * DH * NR                       # 384 DPFP features
NKD = D // 128                           # 8 contraction chunks over d_model
PW = 3 * HPC * DH                        # 384 projection width (q|k|v)

dt = mybir.dt

# chunk storage position: pos = cl*2 + b  (global chunk c = b*NCB + cl)
POS_OF_C = [(c % NCB) * 2 + (c // NCB) for c in range(NCHUNK)]
C_OF_POS = [0] * NCHUNK
for _c, _p in enumerate(POS_OF_C):
    C_OF_POS[_p] = _c


def _install_profshim():
    """Enable NTFF profiling under axon when antenv.axon_hooks is missing."""
    try:
        import antenv
    except ImportError:
        return
    if "antenv.axon_hooks" in sys.modules:
        return
    mod = types.ModuleType("antenv.axon_hooks")
    mod._hook = None
    mod.set_axon_ntff_profile_hook = lambda h: setattr(mod, "_hook", h)
    mod.get_axon_ntff_profile_hook = lambda: mod._hook
    sys.modules["antenv.axon_hooks"] = mod
    antenv.axon_hooks = mod
    try:
        from trn_agent_boot.trn_boot import _ntff_profile_via_ctypes
        mod.set_axon_ntff_profile_hook(
            _ntff_profile_via_ctypes("/opt/axon/libaxon_pjrt.so"))
    except Exception:
        pass


def build_program():
    nc = bacc.Bacc("TRN2", target_bir_lowering=False, debug=False,
                   num_devices=N_CORES)

    # ---- kernel I/O (per-core values supplied via in_maps) ----
    hT_d = nc.declare_dram_parameter("hT", [D, ROWS], dt.bfloat16, isOutput=False)
    wall_d = nc.declare_dram_parameter("wallT", [D, PW], dt.bfloat16, isOutput=False)
    woT_d = nc.declare_dram_parameter("woT", [D, D], dt.bfloat16, isOutput=False)
    hs_d = nc.declare_dram_parameter("h_slice", [RPC, D], dt.float32, isOutput=False)
    mask_d = nc.declare_dram_parameter("mask4", [128, 512], dt.bfloat16, isOutput=False)
    ident_d = nc.declare_dram_parameter("ident", [128, 128], dt.bfloat16, isOutput=False)
    gam_d = nc.declare_dram_parameter("gamma", [1, D], dt.float32, isOutput=False)
    bet_d = nc.declare_dram_parameter("beta", [1, D], dt.float32, isOutput=False)
    out_d = nc.declare_dram_parameter("out", [RPC, D], dt.float32, isOutput=True)

    # internal DRAM bounce buffers: one AllToAll per head
    a2a_in = [nc.dram_tensor(f"a2a_in{h}", [N_CORES, DH, RPC], dt.bfloat16)
              for h in range(HPC)]
    a2a_out = [nc.dram_tensor(f"a2a_out{h}", [N_CORES, DH, RPC], dt.bfloat16)
               for h in range(HPC)]

    with tile.TileContext(nc) as tc:
        with (
            tc.tile_pool(name="const", bufs=1) as Pc,
            tc.tile_pool(name="big", bufs=1) as Pb,
            tc.tile_pool(name="work", bufs=2) as Pw,
            tc.tile_pool(name="ps2", bufs=2, space="PSUM") as Pp,
            tc.tile_pool(name="ps_acc", bufs=1, space="PSUM") as Pacc,
            ExitStack() as _stack,
        ):
            _inner = ExitStack()
            Pi = _inner.enter_context(tc.tile_pool(name="inner", bufs=1))

            # ---------- constants ----------
            mask4 = Pc.tile([128, 512], dt.bfloat16, tag="mask4")
            ident = Pc.tile([128, 128], dt.bfloat16, tag="ident")
            gam = Pc.tile([1, D], dt.float32, tag="gam")
            bet = Pc.tile([1, D], dt.float32, tag="bet")
            ones1 = Pc.tile([1, 128], dt.float32, tag="ones1")
            eps_ln = Pc.tile([128, 1], dt.float32, tag="eps_ln")
            # ones row at partition 64 for the denominator broadcast matmul
            ones65 = Pc.tile([65, 64], dt.bfloat16, tag="ones65")
            nc.vector.memset(eps_ln[:, :], EPS_LN)
            nc.vector.memset(ones65[64:65, :], 1.0)
            nc.sync.dma_start(mask4[:, :], mask_d[:, :])
            nc.sync.dma_start(ident[:, :], ident_d[:, :])
            nc.sync.dma_start(gam[:, :], gam_d[:, :])
            nc.sync.dma_start(bet[:, :], bet_d[:, :])
            nc.vector.memset(ones1[:, :], 1.0)
            # broadcast gamma/beta across partitions via PE (K=1 matmul)
            gam_bc = Pc.tile([128, D], dt.bfloat16, tag="gam_bc")
            bet_bc = Pc.tile([128, D], dt.bfloat16, tag="bet_bc")
            for half in range(2):
                gb_ps = Pp.tile([128, 512], dt.float32, tag="g_ps", bufs=3)
                nc.tensor.matmul(gb_ps[:, :], ones1[:1, :], gam[:1, bass.ts(half, 512)],
                                 start=True, stop=True)
                nc.scalar.copy(gam_bc[:, bass.ts(half, 512)], gb_ps[:, :])
                gb_ps2 = Pp.tile([128, 512], dt.float32, tag="g_ps", bufs=3)
                nc.tensor.matmul(gb_ps2[:, :], ones1[:1, :], bet[:1, bass.ts(half, 512)],
                                 start=True, stop=True)
                nc.scalar.copy(bet_bc[:, bass.ts(half, 512)], gb_ps2[:, :])

            # ---------- persistent big buffers (position-indexed columns) ----------
            # f2_all[p, pos*512 + ht*128 + j]: relu features, ht in (q0,q1,k0,k1)
            f2_all = Pi.tile([128, NCHUNK * 512], dt.bfloat16, tag="f2")
            # va_all[p, pos*130 + h*65 + d]: v augmented with ones column
            va_all = Pb.tile([128, NCHUNK * 130], dt.bfloat16, tag="va")
            # prodT[p, pos*384 + feat] per head-tensor (q0,q1 -> qfT; k0,k1 -> kfT)
            qfT = [Pb.tile([128, NCHUNK * FEAT], dt.bfloat16, tag=f"qfT{i}", name=f"qfT{i}")
                   for i in range(HPC)]
            kfT = [Pb.tile([128, NCHUNK * FEAT], dt.bfloat16, tag=f"kfT{i}", name=f"kfT{i}")
                   for i in range(HPC)]
            # attention output per head, [d, row] layout feeding the A2As
            attn_h = [Pb.tile([64, ROWS], dt.bfloat16, tag=f"attn{h}", name=f"attn{h}")
                      for h in range(HPC)]

            # ones columns of va (exact 1.0)
            va4 = va_all[:, :].rearrange("p (c h d) -> p c h d", h=2, d=65)
            nc.vector.memset(va4[:, :, :, 64:65], 1.0)

            # ---------- phase 1: projections + relu (position order) ----------
            w_sb = Pi.tile([128, NKD * PW], dt.bfloat16, tag="w_sb")
            for kd in range(NKD):
                nc.sync.dma_start(w_sb[:, bass.ts(kd, PW)], wall_d[bass.ts(kd, 128), :])
            # hT loaded as 8 big contiguous DMAs (one per 128-row d_model chunk)
            ht_sb = [Pi.tile([128, ROWS], dt.bfloat16, tag=f"ht{kd}", name=f"ht{kd}")
                     for kd in range(NKD)]
            for kd in range(NKD):
                nc.sync.dma_start(ht_sb[kd][:, :], hT_d[bass.ts(kd, 128), :])

            for pos in range(NCHUNK):
                c = C_OF_POS[pos]
                pps = Pp.tile([128, 512], dt.float32, tag="g_ps", bufs=3)
                for kd in range(NKD):
                    nc.tensor.matmul(pps[:, 0:PW], ht_sb[kd][:, bass.ts(c, 128)],
                                     w_sb[:, bass.ts(kd, PW)],
                                     start=(kd == 0), stop=(kd == NKD - 1))
                # relu(+x) on Scalar, relu(-x) on Vector -> f2 blocks [relu|relu-]
                f2c = f2_all[:, bass.ts(pos, 512)].rearrange("p (b s) -> p b s", b=4, s=128)
                pq = pps[:, 0:256].rearrange("p (b s) -> p b s", b=4, s=64)
                nc.scalar.activation(f2c[:, :, 0:64], pq[:, :, :],
                                     mybir.ActivationFunctionType.Relu)
                nc.vector.tensor_scalar(f2c[:, :, 64:128], pq[:, :, :], -1.0, 0.0,
                                        op0=mybir.AluOpType.mult,
                                        op1=mybir.AluOpType.max)
                # v copy into augmented layout
                vac = va_all[:, bass.ts(pos, 130)].rearrange("p (h d) -> p h d", h=2, d=65)
                pv = pps[:, 256:384].rearrange("p (h d) -> p h d", h=2, d=64)
                nc.scalar.copy(vac[:, :, 0:64], pv[:, :, :])

            # ---------- phase 2: DPFP roll products, two position-groups ----------
            # q0/k0 (head 0) on Vector; q1/k1 (head 1) on GpSimd(Pool)
            for grp in range(2):
                sl = slice(grp * 12, (grp + 1) * 12)
                f2r = f2_all[:, :].rearrange("p (c b j) -> p c b j", b=4, j=128)[:, sl]
                for ht in (0, 2, 1, 3):              # q0 k0 (DVE) then q1 k1 (Pool)
                    eng = nc.vector if ht in (0, 2) else nc.gpsimd
                    dst = (qfT if ht < 2 else kfT)[ht % 2]
                    dstr = dst[:, :].rearrange("p (c t j) -> p c t j", t=NR, j=128)[:, sl]
                    for t in range(1, NR + 1):
                        eng.tensor_mul(dstr[:, :, t - 1, t:128],
                                       f2r[:, :, ht, t:128],
                                       f2r[:, :, ht, 0:128 - t])
                        eng.tensor_mul(dstr[:, :, t - 1, 0:t],
                                       f2r[:, :, ht, 0:t],
                                       f2r[:, :, ht, 128 - t:128])

            _inner.close()   # frees hT / f2 / w_sb SBUF space before attention

            # ---------- phase-4 persistent buffers + weight prefetch ----------
            Po = _stack.enter_context(tc.tile_pool(name="post", bufs=1))
            # wo2[h]: rank-pair-stacked Wo rows: partitions 0-63 <- rank 2q,
            # 64-127 <- rank 2q+1 (full-K o-projection matmuls)
            wo2 = [Po.tile([128, 4 * D], dt.bfloat16, tag=f"wo{h}", name=f"wo{h}")
                   for h in range(HPC)]
            for h in range(HPC):
                for q in range(4):
                    nc.sync.dma_start(
                        wo2[h][0:64, bass.ts(q, D)],
                        woT_d[(2 * q) * 128 + h * 64:(2 * q) * 128 + (h + 1) * 64, :])
                    nc.sync.dma_start(
                        wo2[h][64:128, bass.ts(q, D)],
                        woT_d[(2 * q + 1) * 128 + h * 64:(2 * q + 1) * 128 + (h + 1) * 64, :])
            hs_sb = [Po.tile([128, D], dt.float32, tag=f"hs{rc}", name=f"hs{rc}")
                     for rc in range(3)]
            for rc in range(3):
                nc.sync.dma_start(hs_sb[rc][:, :], hs_d[bass.ts(rc, 128), :])
            x_sb = [Po.tile([128, D], dt.float32, tag=f"x{rc}", name=f"x{rc}")
                    for rc in range(3)]
            # asl2[h]: rank-pair-stacked gathered attention (partitions as wo2)
            asl2 = [Po.tile([128, 4 * RPC], dt.bfloat16, tag=f"asl{h}", name=f"asl{h}")
                    for h in range(HPC)]

            # ---------- phase 3: attention, head-outer; A2A per head ----------
            for h in range(HPC):
                kv_acc = Pacc.tile([128, 390], dt.float32, tag="kvp", name=f"kvp{h}")
                kv_sb = None
                for cl in range(NCB):
                    # per-batch transposed feature chunks [feat, i] via PE transpose
                    qf_sb, kf_sb = [], []
                    for b in range(BATCH):
                        pos = cl * 2 + b
                        tq = Pw.tile([128, FEAT], dt.bfloat16, tag="qf_c", bufs=4)
                        tk = Pw.tile([128, FEAT], dt.bfloat16, tag="kf_c", bufs=4)
                        psq = Pp.tile([128, 512], dt.bfloat16, tag="g_ps", bufs=3)
                        psk = Pp.tile([128, 512], dt.bfloat16, tag="g_ps", bufs=3)
                        for t in range(NR):
                            nc.tensor.transpose(
                                psq[:, bass.ts(t, 128)],
                                qfT[h][:, pos * FEAT + t * 128:pos * FEAT + (t + 1) * 128],
                                ident[:, :])
                            nc.tensor.transpose(
                                psk[:, bass.ts(t, 128)],
                                kfT[h][:, pos * FEAT + t * 128:pos * FEAT + (t + 1) * 128],
                                ident[:, :])
                        nc.scalar.copy(tq[:, :], psq[:, 0:FEAT])
                        nc.vector.tensor_copy(tk[:, :], psk[:, 0:FEAT])
                        qf_sb.append(tq)
                        kf_sb.append(tk)

                    # scoreT[j, i] both batches in one PSUM bank
                    sc_ps = Pp.tile([128, 512], dt.float32, tag="sc_ps", bufs=1)
                    for b in range(BATCH):
                        for t in range(NR):
                            nc.tensor.matmul(sc_ps[:, bass.ts(b, 128)],
                                             kf_sb[b][:, bass.ts(t, 128)],
                                             qf_sb[b][:, bass.ts(t, 128)],
                                             start=(t == 0), stop=(t == NR - 1))
                    probT = Pw.tile([128, 256], dt.bfloat16, tag="probT")
                    nc.vector.tensor_mul(probT[:, :], sc_ps[:, 0:256], mask4[:, 0:256])

                    # uT[d, i]: d 0-63 = unnorm attn, d 64 = denom; intra + state
                    # (padded to 512 f32 cols = full 2KB bank: matmul start=True
                    # zeroes the whole bank, so no other tile may share it)
                    uT_full = Pp.tile([65, 512], dt.float32, tag="uT", bufs=2)
                    uT = uT_full[:, 0:256]
                    for b in range(BATCH):
                        pos = cl * 2 + b
                        va_c = va_all[:, pos * 130 + h * 65:pos * 130 + (h + 1) * 65]
                        nc.tensor.matmul(uT[:, bass.ts(b, 128)], va_c,
                                         probT[:, bass.ts(b, 128)],
                                         start=True, stop=(cl == 0))
                        if cl > 0:
                            for t in range(NR):
                                nc.tensor.matmul(uT[:, bass.ts(b, 128)],
                                                 kv_sb[b][:, bass.ts(t, 65)],
                                                 qf_sb[b][:, bass.ts(t, 128)],
                                                 start=False, stop=(t == NR - 1))

                    # KV state update: KV += kfT_c.T @ va_c  (PSUM accumulator)
                    kv_pk = Pw.tile([128, 390], dt.bfloat16, tag="kv_pk", bufs=2)
                    kv_sb_next = [kv_pk[:, bass.ts(b, 195)] for b in range(BATCH)]
                    for b in range(BATCH):
                        pos = cl * 2 + b
                        va_c = va_all[:, pos * 130 + h * 65:pos * 130 + (h + 1) * 65]
                        for t in range(NR):
                            # start only on the very first touch of this bank
                            # (start marks the whole 2KB zero region pending)
                            nc.tensor.matmul(
                                kv_acc[:, b * 195 + t * 65:b * 195 + (t + 1) * 65],
                                kfT[h][:, pos * FEAT + t * 128:pos * FEAT + (t + 1) * 128],
                                va_c,
                                start=(cl == 0 and b == 0 and t == 0),
                                stop=(cl == NCB - 1),
                                skip_group_check=True)
                    if cl < NCB - 1:
                        nc.scalar.copy(kv_pk[:, :], kv_acc[:, :])
                    kv_sb = kv_sb_next

                    # normalize: attn[d, i] = uT[d, i] / (uT[64, i] + eps)
                    # denom row -> SBUF (+eps), K=1 PE broadcast, recip, mul
                    dsb = Pw.tile([65, 256], dt.bfloat16, tag="dsb")
                    nc.vector.tensor_scalar_add(dsb[64:65, :], uT[64:65, :], EPS_D)
                    bc_full = Pp.tile([64, 512], dt.float32, tag="bc", bufs=1)
                    bc = bc_full[:, 0:256]
                    nc.tensor.matmul(bc[:, :], ones65[64:65, :], dsb[64:65, :],
                                     start=True, stop=True)
                    rec = Pw.tile([64, 256], dt.float32, tag="rec")
                    nc.vector.reciprocal(rec[:, :], bc[:, :])
                    attn_view = attn_h[h][:, :].rearrange(
                        "p (b s) -> p b s", b=2, s=SEQ)[:, :, cl * 128:(cl + 1) * 128]
                    nc.vector.tensor_mul(
                        attn_view,
                        uT[0:64, :].rearrange("p (b i) -> p b i", b=2, i=128),
                        rec[:, :].rearrange("p (b i) -> p b i", b=2, i=128))

                # ---- AllToAll for this head ----
                for r in range(N_CORES):
                    nc.sync.dma_start(a2a_in[h][r, :, :],
                                      attn_h[h][:, bass.ts(r, RPC)])
                nc.gpsimd.collective_compute(
                    "AllToAll", mybir.AluOpType.bypass,
                    replica_groups=[list(range(N_CORES))],
                    ins=[a2a_in[h].ap().opt()], outs=[a2a_out[h].ap().opt()])

            # ---------- phase 4a: head-0 partial o-projection (hides A2A#1) ----
            for h in range(HPC):
                for q in range(4):
                    nc.sync.dma_start(asl2[h][0:64, bass.ts(q, RPC)],
                                      a2a_out[h][2 * q, :, :])
                    nc.sync.dma_start(asl2[h][64:128, bass.ts(q, RPC)],
                                      a2a_out[h][2 * q + 1, :, :])
                for rc in range(3):
                    for n in range(2):
                        ops = Pp.tile([128, 512], dt.float32, tag="g_ps", bufs=3)
                        for q in range(4):
                            nc.tensor.matmul(
                                ops[:, :],
                                asl2[h][:, q * RPC + rc * 128:q * RPC + (rc + 1) * 128],
                                wo2[h][:, q * D + n * 512:q * D + (n + 1) * 512],
                                start=(q == 0), stop=(q == 3))
                        if h == 0:
                            # x = attn0-part + h_slice
                            nc.vector.scalar_tensor_tensor(
                                x_sb[rc][:, bass.ts(n, 512)], ops[:, :], 0.0,
                                hs_sb[rc][:, bass.ts(n, 512)],
                                op0=mybir.AluOpType.add, op1=mybir.AluOpType.add)
                        else:
                            # xf = attn1-part + x ; accumulate row-sum for mean
                            xf = _XF[rc]
                            nc.vector.scalar_tensor_tensor(
                                xf[:, bass.ts(n, 512)], ops[:, :], 0.0,
                                x_sb[rc][:, bass.ts(n, 512)],
                                op0=mybir.AluOpType.add, op1=mybir.AluOpType.add,
                                accum_out=_S2[rc][:, n:n + 1])
                if h == 0:
                    # allocate LN scratch between the two passes
                    _XF = [Po.tile([128, D], dt.float32, tag=f"xf{rc}", name=f"xf{rc}")
                           for rc in range(3)]
                    _S2 = [Pw.tile([128, 2], dt.float32, tag=f"s2_{rc}",
                                   name=f"s2_{rc}") for rc in range(3)]

            # ---------- phase 4b: layernorm + store ----------
            for rc in range(3):
                xf = _XF[rc]
                s2 = _S2[rc]
                mean = Pw.tile([128, 1], dt.float32, tag="mean")
                nc.vector.tensor_reduce(mean[:, :], s2[:, :],
                                        axis=mybir.AxisListType.X,
                                        op=mybir.AluOpType.add)
                nc.vector.tensor_scalar_mul(mean[:, :], mean[:, :], 1.0 / D)
                var = Pw.tile([128, 1], dt.float32, tag="var")
                nc.vector.tensor_scalar(xf[:, :], xf[:, :], mean[:, :], None,
                                        op0=mybir.AluOpType.subtract)
                sq = Po.tile([128, D], dt.float32, tag="sq", bufs=2)
                nc.vector.tensor_mul(sq[:, :], xf[:, :], xf[:, :])
                nc.vector.tensor_reduce(var[:, :], sq[:, :],
                                        axis=mybir.AxisListType.X,
                                        op=mybir.AluOpType.add)
                # rstd = 1/sqrt(var/D + eps)
                rstd = Pw.tile([128, 1], dt.float32, tag="rstd")
                nc.scalar.activation(rstd[:, :], var[:, :],
                                     mybir.ActivationFunctionType.Sqrt,
                                     bias=eps_ln[:, :], scale=1.0 / D)
                nc.vector.reciprocal(rstd[:, :], rstd[:, :])
                # y = (xc * rstd) * gamma + beta   (reuse sq as output buf)
                nc.vector.scalar_tensor_tensor(
                    sq[:, :], xf[:, :], rstd[:, :], gam_bc[:, :],
                    op0=mybir.AluOpType.mult, op1=mybir.AluOpType.mult)
                nc.vector.tensor_add(sq[:, :], sq[:, :], bet_bc[:, :])
                nc.sync.dma_start(out_d[bass.ts(rc, 128), :], sq[:, :])

    nc.finalize()
    return nc


_PROGRAM = None


def _get_program():
    global _PROGRAM
    if _PROGRAM is None:
        _PROGRAM = build_program()
    return _PROGRAM


def _host_prep(h, Wq, Wkv, Wo, ln_gamma, ln_beta):
    h = np.asarray(h, F32)
    h_bm = np.ascontiguousarray(h.transpose(1, 0, 2).reshape(ROWS, D))
    hT = np.ascontiguousarray(h_bm.T).astype(BF16)
    Wq_h = np.asarray(Wq, F32).reshape(NH, DH, D)
    Wk_h = np.asarray(Wkv, F32)[:NH * DH].reshape(NH, DH, D)
    Wv_h = np.asarray(Wkv, F32)[NH * DH:].reshape(NH, DH, D)
    WoT = np.ascontiguousarray(np.asarray(Wo, F32).T).astype(BF16)
    mask4 = np.tile(np.triu(np.ones((128, 128), F32)), (1, 4)).astype(BF16)
    ident = np.eye(128, dtype=F32).astype(BF16)
    gamma = np.asarray(ln_gamma, F32).reshape(1, D)
    beta = np.asarray(ln_beta, F32).reshape(1, D)

    in_maps = []
    for core in range(N_CORES):
        hh = [HPC * core + i for i in range(HPC)]
        W_all = np.concatenate([
            np.concatenate([Wq_h[j] * S_FOLD for j in hh]),
            np.concatenate([Wk_h[j] for j in hh]),
            np.concatenate([Wv_h[j] for j in hh]),
        ])
        in_maps.append({
            "hT": hT,
            "wallT": np.ascontiguousarray(W_all.T).astype(BF16),
            "woT": WoT,
            "h_slice": np.ascontiguousarray(h_bm[core * RPC:(core + 1) * RPC]),
            "mask4": mask4,
            "ident": ident,
            "gamma": gamma,
            "beta": beta,
        })
    return in_maps


def run(inputs, trace=False):
    """Run on hardware; returns (output [SEQ,BATCH,D] f32, BassKernelResults)."""
    _install_profshim()
    nc = _get_program()
    in_maps = _host_prep(inputs["h"], inputs["Wq"], inputs["Wkv"], inputs["Wo"],
                         inputs["ln_gamma"], inputs["ln_beta"])
    res = run_bass_kernel_spmd(nc, in_maps, core_ids=list(range(N_CORES)),
                               trace=trace)
    out_bm = np.concatenate([res.results[c]["out"] for c in range(N_CORES)], axis=0)
    out = out_bm.reshape(BATCH, SEQ, D).transpose(1, 0, 2).astype(F32)
    return np.ascontiguousarray(out), res


def kernel(**inputs):
    out, _ = run(inputs, trace=False)
    return out


# revision 8
# speedup vs baseline: 1.0534x; 1.0534x over previous
"""Trainium2 Bass kernel for nn_MemTransformerLM (DPFP linear-attention block).

Full inputs in, full output out. Internally: head-shards across 8 NeuronCores
(2 heads/core), runs causal linear attention as a chunked prefix-sum (the
reference's sum-normalized kernelized attention factorizes: no SxS score
materialization), AllToAll re-shards heads->rows for the output projection,
and each core LayerNorms its row slice. Host concatenates the 8 row slices.

v2 structure (vs v1):
- per-head attention buffers so head-0's AllToAll stores don't falsely
  depend on head-1's copies (partition aliasing in dep tracking)
- attention emits u^T [d, i] directly (intra term va^T @ probT, state term
  KV^T @ qf) so no output transpose / scalar copy; normalization uses a
  K=1 PE broadcast of the reciprocal denominator row
- o-projection split per head: head-0's partial (x = hs + attn0 @ Wo0)
  runs while head-1's AllToAll is in flight, keeping the PE warm through
  the collective and leaving only head-1's partial + LayerNorm on the tail
- o-projection stacks rank pairs to use all 128 PE rows (K=128 not 64)
- phase-1/2 buffers freed before attention so phase-4 weights stay resident
- DPFP products split across Vector and GpSimd(Pool) engines
"""
import os
import sys
import types
from contextlib import ExitStack

for _p in ("/opt/trn_rl_repo",):
    if _p not in sys.path:
        sys.path.insert(0, _p)

import numpy as np
import ml_dtypes

import concourse.bass as bass
import concourse.mybir as mybir
import concourse.tile as tile
from concourse import bacc
from concourse.bass_utils import run_bass_kernel_spmd

BF16 = ml_dtypes.bfloat16
F32 = np.float32

SEQ, BATCH, D = 1536, 2, 1024
NH, DH, NR = 16, 64, 3
SCALE = 1.0 / float(np.sqrt(DH))
S_FOLD = float(np.sqrt(SCALE))           # folded into Wq rows (squared by DPFP products)
EPS_D, EPS_LN = 1e-5, 1e-5
N_CORES = 8
HPC = NH // N_CORES                      # heads per core (2)
ROWS = SEQ * BATCH                       # 3072 batch-major rows
RPC = ROWS // N_CORES                    # 384 output rows per core
NCHUNK = ROWS // 128                     # 24 chunks of 128 rows
NCB = NCHUNK // BATCH                    # 12 chunks per batch
FEAT = 2 * DH * NR                       # 384 DPFP features
NKD = D // 128                           # 8 contraction chunks over d_model
PW = 3 * HPC * DH                        # 384 projection width (q|k|v)

dt = mybir.dt

# chunk storage position: pos = cl*2 + b  (global chunk c = b*NCB + cl)
POS_OF_C = [(c % NCB) * 2 + (c // NCB) for c in range(NCHUNK)]
C_OF_POS = [0] * NCHUNK
for _c, _p in enumerate(POS_OF_C):
    C_OF_POS[_p] = _c


def _install_profshim():
    """Enable NTFF profiling under axon when antenv.axon_hooks is missing."""
    try:
        import antenv
    except ImportError:
        return
    if "antenv.axon_hooks" in sys.modules:
        return
    mod = types.ModuleType("antenv.axon_hooks")
    mod._hook = None
    mod.set_axon_ntff_profile_hook = lambda h: setattr(mod, "_hook", h)
    mod.get_axon_ntff_profile_hook = lambda: mod._hook
    sys.modules["antenv.axon_hooks"] = mod
    antenv.axon_hooks = mod
    try:
        from trn_agent_boot.trn_boot import _ntff_profile_via_ctypes
        mod.set_axon_ntff_profile_hook(
            _ntff_profile_via_ctypes("/opt/axon/libaxon_pjrt.so"))
    except Exception:
        pass


def build_program():
    nc = bacc.Bacc("TRN2", target_bir_lowering=False, debug=False,
                   num_devices=N_CORES)

    # ---- kernel I/O (per-core values supplied via in_maps) ----
    hT_d = nc.declare_dram_parameter("hT", [D, ROWS], dt.bfloat16, isOutput=False)
    wall_d = nc.declare_dram_parameter("wallT", [D, PW], dt.bfloat16, isOutput=False)
    woT_d = nc.declare_dram_parameter("woT", [D, D], dt.bfloat16, isOutput=False)
    hs_d = nc.declare_dram_parameter("h_slice", [RPC, D], dt.float32, isOutput=False)
    mask_d = nc.declare_dram_parameter("mask4", [128, 512], dt.bfloat16, isOutput=False)
    ident_d = nc.declare_dram_parameter("ident", [128, 128], dt.bfloat16, isOutput=False)
    gam_d = nc.declare_dram_parameter("gamma", [1, D], dt.float32, isOutput=False)
    bet_d = nc.declare_dram_parameter("beta", [1, D], dt.float32, isOutput=False)
    out_d = nc.declare_dram_parameter("out", [RPC, D], dt.float32, isOutput=True)

    # internal DRAM bounce buffers: one AllToAll per head
    a2a_in = [nc.dram_tensor(f"a2a_in{h}", [N_CORES, DH, RPC], dt.bfloat16)
              for h in range(HPC)]
    a2a_out = [nc.dram_tensor(f"a2a_out{h}", [N_CORES, DH, RPC], dt.bfloat16)
               for h in range(HPC)]

    with tile.TileContext(nc) as tc:
        with (
            tc.tile_pool(name="const", bufs=1) as Pc,
            tc.tile_pool(name="big", bufs=1) as Pb,
            tc.tile_pool(name="work", bufs=2) as Pw,
            tc.tile_pool(name="ps2", bufs=2, space="PSUM") as Pp,
            tc.tile_pool(name="ps_acc", bufs=1, space="PSUM") as Pacc,
            ExitStack() as _stack,
        ):
            _inner = ExitStack()
            Pi = _inner.enter_context(tc.tile_pool(name="inner", bufs=1))

            # ---------- constants ----------
            mask4 = Pc.tile([128, 512], dt.bfloat16, tag="mask4")
            ident = Pc.tile([128, 128], dt.bfloat16, tag="ident")
            gam = Pc.tile([1, D], dt.float32, tag="gam")
            bet = Pc.tile([1, D], dt.float32, tag="bet")
            ones1 = Pc.tile([1, 128], dt.float32, tag="ones1")
            eps_ln = Pc.tile([128, 1], dt.float32, tag="eps_ln")
            # ones row at partition 64 for the denominator broadcast matmul
            ones65 = Pc.tile([65, 64], dt.bfloat16, tag="ones65")
            eps65 = Pc.tile([65, 1], dt.float32, tag="eps65")
            nc.vector.memset(eps_ln[:, :], EPS_LN)
            nc.vector.memset(ones65[64:65, :], 1.0)
            nc.vector.memset(eps65[:, :], EPS_D)
            nc.sync.dma_start(mask4[:, :], mask_d[:, :])
            nc.sync.dma_start(ident[:, :], ident_d[:, :])
            nc.sync.dma_start(gam[:, :], gam_d[:, :])
            nc.sync.dma_start(bet[:, :], bet_d[:, :])
            nc.vector.memset(ones1[:, :], 1.0)
            # broadcast gamma/beta across partitions via PE (K=1 matmul)
            gam_bc = Pc.tile([128, D], dt.bfloat16, tag="gam_bc")
            bet_bc = Pc.tile([128, D], dt.bfloat16, tag="bet_bc")
            for half in range(2):
                gb_ps = Pp.tile([128, 512], dt.float32, tag="g_ps", bufs=4)
                nc.tensor.matmul(gb_ps[:, :], ones1[:1, :], gam[:1, bass.ts(half, 512)],
                                 start=True, stop=True)
                nc.scalar.copy(gam_bc[:, bass.ts(half, 512)], gb_ps[:, :])
                gb_ps2 = Pp.tile([128, 512], dt.float32, tag="g_ps", bufs=4)
                nc.tensor.matmul(gb_ps2[:, :], ones1[:1, :], bet[:1, bass.ts(half, 512)],
                                 start=True, stop=True)
                nc.scalar.copy(bet_bc[:, bass.ts(half, 512)], gb_ps2[:, :])

            # ---------- persistent big buffers (position-indexed columns) ----------
            # f2_all[p, pos*512 + ht*128 + j]: relu features, ht in (q0,q1,k0,k1)
            f2_all = Pi.tile([128, NCHUNK * 512], dt.bfloat16, tag="f2")
            # va_all[p, pos*130 + h*65 + d]: v augmented with ones column
            va_all = Pb.tile([128, NCHUNK * 130], dt.bfloat16, tag="va")
            # prodT[p, pos*384 + feat] per head-tensor (q0,q1 -> qfT; k0,k1 -> kfT)
            qfT = [Pb.tile([128, NCHUNK * FEAT], dt.bfloat16, tag=f"qfT{i}", name=f"qfT{i}")
                   for i in range(HPC)]
            kfT = [Pb.tile([128, NCHUNK * FEAT], dt.bfloat16, tag=f"kfT{i}", name=f"kfT{i}")
                   for i in range(HPC)]
            # attention output per head, [d, row] layout feeding the A2As
            attn_h = [Pb.tile([64, ROWS], dt.bfloat16, tag=f"attn{h}", name=f"attn{h}")
                      for h in range(HPC)]

            # ones columns of va (exact 1.0)
            va4 = va_all[:, :].rearrange("p (c h d) -> p c h d", h=2, d=65)
            nc.vector.memset(va4[:, :, :, 64:65], 1.0)

            # ---------- phase 1: projections + relu (position order) ----------
            w_sb = Pi.tile([128, NKD * PW], dt.bfloat16, tag="w_sb")
            for kd in range(NKD):
                nc.sync.dma_start(w_sb[:, bass.ts(kd, PW)], wall_d[bass.ts(kd, 128), :])
            # hT loaded as 8 big contiguous DMAs (one per 128-row d_model chunk)
            ht_sb = [Pi.tile([128, ROWS], dt.bfloat16, tag=f"ht{kd}", name=f"ht{kd}")
                     for kd in range(NKD)]
            for kd in range(NKD):
                nc.sync.dma_start(ht_sb[kd][:, :], hT_d[bass.ts(kd, 128), :])

            for pos in range(NCHUNK):
                c = C_OF_POS[pos]
                pps = Pp.tile([128, 512], dt.float32, tag="g_ps", bufs=4)
                for kd in range(NKD):
                    nc.tensor.matmul(pps[:, 0:PW], ht_sb[kd][:, bass.ts(c, 128)],
                                     w_sb[:, bass.ts(kd, PW)],
                                     start=(kd == 0), stop=(kd == NKD - 1))
                # relu(+x) on Scalar, relu(-x) on Vector -> f2 blocks [relu|relu-]
                f2c = f2_all[:, bass.ts(pos, 512)].rearrange("p (b s) -> p b s", b=4, s=128)
                pq = pps[:, 0:256].rearrange("p (b s) -> p b s", b=4, s=64)
                nc.scalar.activation(f2c[:, :, 0:64], pq[:, :, :],
                                     mybir.ActivationFunctionType.Relu)
                nc.scalar.activation(f2c[:, :, 64:128], pq[:, :, :],
                                     mybir.ActivationFunctionType.Relu, scale=-1.0)
                # v copy into augmented layout
                vac = va_all[:, bass.ts(pos, 130)].rearrange("p (h d) -> p h d", h=2, d=65)
                pv = pps[:, 256:384].rearrange("p (h d) -> p h d", h=2, d=64)
                nc.scalar.copy(vac[:, :, 0:64], pv[:, :, :])

            # ---------- phase 2: DPFP roll products, two position-groups ----------
            # q0/k0 (head 0) on Vector; q1/k1 (head 1) on GpSimd(Pool)
            for grp in range(2):
                sl = slice(grp * 12, (grp + 1) * 12)
                f2r = f2_all[:, :].rearrange("p (c b j) -> p c b j", b=4, j=128)[:, sl]
                for ht in (0, 2, 1, 3):              # q0 k0 first (head 0 unblocks)
                    eng = nc.vector
                    dst = (qfT if ht < 2 else kfT)[ht % 2]
                    dstr = dst[:, :].rearrange("p (c t j) -> p c t j", t=NR, j=128)[:, sl]
                    for t in range(1, NR + 1):
                        eng.tensor_mul(dstr[:, :, t - 1, t:128],
                                       f2r[:, :, ht, t:128],
                                       f2r[:, :, ht, 0:128 - t])
                        eng.tensor_mul(dstr[:, :, t - 1, 0:t],
                                       f2r[:, :, ht, 0:t],
                                       f2r[:, :, ht, 128 - t:128])

            _inner.close()   # frees hT / f2 / w_sb SBUF space before attention

            # ---------- phase-4 persistent buffers + weight prefetch ----------
            Po = _stack.enter_context(tc.tile_pool(name="post", bufs=1))
            # wo2[h]: rank-pair-stacked Wo rows: partitions 0-63 <- rank 2q,
            # 64-127 <- rank 2q+1 (full-K o-projection matmuls)
            wo2 = [Po.tile([128, 4 * D], dt.bfloat16, tag=f"wo{h}", name=f"wo{h}")
                   for h in range(HPC)]
            for h in range(HPC):
                for q in range(4):
                    nc.sync.dma_start(
                        wo2[h][0:64, bass.ts(q, D)],
                        woT_d[(2 * q) * 128 + h * 64:(2 * q) * 128 + (h + 1) * 64, :])
                    nc.sync.dma_start(
                        wo2[h][64:128, bass.ts(q, D)],
                        woT_d[(2 * q + 1) * 128 + h * 64:(2 * q + 1) * 128 + (h + 1) * 64, :])
            hs_sb = [Po.tile([128, D], dt.float32, tag=f"hs{rc}", name=f"hs{rc}")
                     for rc in range(3)]
            for rc in range(3):
                nc.sync.dma_start(hs_sb[rc][:, :], hs_d[bass.ts(rc, 128), :])
            x_sb = [Po.tile([128, D], dt.float32, tag=f"x{rc}", name=f"x{rc}")
                    for rc in range(3)]
            # asl2[h]: rank-pair-stacked gathered attention (partitions as wo2)
            asl2 = [Po.tile([128, 4 * RPC], dt.bfloat16, tag=f"asl{h}", name=f"asl{h}")
                    for h in range(HPC)]

            # ---------- phase 3: attention, head-outer; A2A per head ----------
            for h in range(HPC):
                kv_acc = Pacc.tile([128, 390], dt.float32, tag="kvp", name=f"kvp{h}")
                kv_sb = None
                for cl in range(NCB):
                    # per-batch transposed feature chunks [feat, i] via PE transpose
                    qf_sb, kf_sb = [], []
                    for b in range(BATCH):
                        pos = cl * 2 + b
                        tq = Pw.tile([128, FEAT], dt.bfloat16, tag="qf_c", bufs=4)
                        tk = Pw.tile([128, FEAT], dt.bfloat16, tag="kf_c", bufs=4)
                        psq = Pp.tile([128, 512], dt.bfloat16, tag="g_ps", bufs=4)
                        psk = Pp.tile([128, 512], dt.bfloat16, tag="g_ps", bufs=4)
                        for t in range(NR):
                            nc.tensor.transpose(
                                psq[:, bass.ts(t, 128)],
                                qfT[h][:, pos * FEAT + t * 128:pos * FEAT + (t + 1) * 128],
                                ident[:, :])
                            nc.tensor.transpose(
                                psk[:, bass.ts(t, 128)],
                                kfT[h][:, pos * FEAT + t * 128:pos * FEAT + (t + 1) * 128],
                                ident[:, :])
                        nc.scalar.copy(tq[:, :], psq[:, 0:FEAT])
                        nc.vector.tensor_copy(tk[:, :], psk[:, 0:FEAT])
                        qf_sb.append(tq)
                        kf_sb.append(tk)

                    # scoreT[j, i] both batches in one PSUM bank
                    sc_ps = Pp.tile([128, 512], dt.float32, tag="sc_ps", bufs=1)
                    for b in range(BATCH):
                        for t in range(NR):
                            nc.tensor.matmul(sc_ps[:, bass.ts(b, 128)],
                                             kf_sb[b][:, bass.ts(t, 128)],
                                             qf_sb[b][:, bass.ts(t, 128)],
                                             start=(t == 0), stop=(t == NR - 1))
                    probT = Pw.tile([128, 256], dt.bfloat16, tag="probT")
                    nc.vector.tensor_mul(probT[:, :], sc_ps[:, 0:256], mask4[:, 0:256])

                    # uT[d, i]: d 0-63 = unnorm attn, d 64 = denom; intra + state
                    # (padded to 512 f32 cols = full 2KB bank: matmul start=True
                    # zeroes the whole bank, so no other tile may share it)
                    uT_full = Pp.tile([65, 512], dt.float32, tag="uT", bufs=2)
                    uT = uT_full[:, 0:256]
                    for b in range(BATCH):
                        pos = cl * 2 + b
                        va_c = va_all[:, pos * 130 + h * 65:pos * 130 + (h + 1) * 65]
                        nc.tensor.matmul(uT[:, bass.ts(b, 128)], va_c,
                                         probT[:, bass.ts(b, 128)],
                                         start=True, stop=(cl == 0))
                        if cl > 0:
                            for t in range(NR):
                                nc.tensor.matmul(uT[:, bass.ts(b, 128)],
                                                 kv_sb[b][:, bass.ts(t, 65)],
                                                 qf_sb[b][:, bass.ts(t, 128)],
                                                 start=False, stop=(t == NR - 1))

                    # KV state update: KV += kfT_c.T @ va_c  (PSUM accumulator)
                    kv_pk = Pw.tile([128, 390], dt.bfloat16, tag="kv_pk", bufs=2)
                    kv_sb_next = [kv_pk[:, bass.ts(b, 195)] for b in range(BATCH)]
                    for b in range(BATCH):
                        pos = cl * 2 + b
                        va_c = va_all[:, pos * 130 + h * 65:pos * 130 + (h + 1) * 65]
                        for t in range(NR):
                            # start only on the very first touch of this bank
                            # (start marks the whole 2KB zero region pending)
                            nc.tensor.matmul(
                                kv_acc[:, b * 195 + t * 65:b * 195 + (t + 1) * 65],
                                kfT[h][:, pos * FEAT + t * 128:pos * FEAT + (t + 1) * 128],
                                va_c,
                                start=(cl == 0 and b == 0 and t == 0),
                                stop=(cl == NCB - 1),
                                skip_group_check=True)
                    if cl < NCB - 1:
                        nc.scalar.copy(kv_pk[:, :], kv_acc[:, :])
                    kv_sb = kv_sb_next

                    # normalize: attn[d, i] = uT[d, i] / (uT[64, i] + eps)
                    # denom row -> SBUF (+eps on ACT), K=1 PE broadcast into the
                    # dead score bank, fast recip, per-batch mul (contiguous
                    # writes keep the A2A store deps precise)
                    dsb = Pw.tile([65, 256], dt.bfloat16, tag="dsb")
                    nc.scalar.activation(dsb[64:65, :], uT[64:65, :],
                                         mybir.ActivationFunctionType.Copy,
                                         bias=EPS_D)
                    bc_full = Pp.tile([64, 512], dt.float32, tag="sc_ps", bufs=1)
                    bc = bc_full[:, 0:256]
                    nc.tensor.matmul(bc[:, :], ones65[64:65, :], dsb[64:65, :],
                                     start=True, stop=True)
                    rec = Pw.tile([64, 256], dt.float32, tag="rec")
                    nc.vector.reciprocal_approx_fast(rec[:, :], bc[:, :])
                    for b in range(BATCH):
                        nc.vector.tensor_mul(
                            attn_h[h][:, b * SEQ + cl * 128:b * SEQ + (cl + 1) * 128],
                            uT[0:64, bass.ts(b, 128)],
                            rec[:, bass.ts(b, 128)])

                # ---- AllToAll for this head ----
                for r in range(N_CORES):
                    nc.sync.dma_start(a2a_in[h][r, :, :],
                                      attn_h[h][:, bass.ts(r, RPC)])
                nc.gpsimd.collective_compute(
                    "AllToAll", mybir.AluOpType.bypass,
                    replica_groups=[list(range(N_CORES))],
                    ins=[a2a_in[h].ap().opt()], outs=[a2a_out[h].ap().opt()])

            # ---------- phase 4a: head-0 partial o-projection (hides A2A#1) ----
            for h in range(HPC):
                for q in range(4):
                    nc.sync.dma_start(asl2[h][0:64, bass.ts(q, RPC)],
                                      a2a_out[h][2 * q, :, :])
                    nc.sync.dma_start(asl2[h][64:128, bass.ts(q, RPC)],
                                      a2a_out[h][2 * q + 1, :, :])
                for rc in range(3):
                    for n in range(2):
                        ops = Pp.tile([128, 512], dt.float32, tag="g_ps", bufs=4)
                        for q in range(4):
                            nc.tensor.matmul(
                                ops[:, :],
                                asl2[h][:, q * RPC + rc * 128:q * RPC + (rc + 1) * 128],
                                wo2[h][:, q * D + n * 512:q * D + (n + 1) * 512],
                                start=(q == 0), stop=(q == 3))
                        if h == 0:
                            # x = attn0-part + h_slice
                            nc.vector.scalar_tensor_tensor(
                                x_sb[rc][:, bass.ts(n, 512)], ops[:, :], 0.0,
                                hs_sb[rc][:, bass.ts(n, 512)],
                                op0=mybir.AluOpType.add, op1=mybir.AluOpType.add)
                        else:
                            # xf = attn1-part + x ; accumulate row-sum for mean
                            xf = _XF[rc]
                            nc.vector.scalar_tensor_tensor(
                                xf[:, bass.ts(n, 512)], ops[:, :], 0.0,
                                x_sb[rc][:, bass.ts(n, 512)],
                                op0=mybir.AluOpType.add, op1=mybir.AluOpType.add,
                                accum_out=_S2[rc][:, n:n + 1])
                if h == 0:
                    # allocate LN scratch between the two passes
                    _XF = [Po.tile([128, D], dt.float32, tag=f"xf{rc}", name=f"xf{rc}")
                           for rc in range(3)]
                    _S2 = [Pw.tile([128, 2], dt.float32, tag=f"s2_{rc}",
                                   name=f"s2_{rc}") for rc in range(3)]

            # ---------- phase 4b: layernorm + store ----------
            for rc in range(3):
                xf = _XF[rc]
                s2 = _S2[rc]
                mean = Pw.tile([128, 1], dt.float32, tag="mean")
                nc.vector.tensor_reduce(mean[:, :], s2[:, :],
                                        axis=mybir.AxisListType.X,
                                        op=mybir.AluOpType.add)
                nc.vector.tensor_scalar_mul(mean[:, :], mean[:, :], 1.0 / D)
                var = Pw.tile([128, 1], dt.float32, tag="var")
                nc.vector.tensor_scalar(xf[:, :], xf[:, :], mean[:, :], None,
                                        op0=mybir.AluOpType.subtract)
                sq = Po.tile([128, D], dt.float32, tag="sq", bufs=2)
                nc.vector.tensor_mul(sq[:, :], xf[:, :], xf[:, :])
                nc.vector.tensor_reduce(var[:, :], sq[:, :],
                                        axis=mybir.AxisListType.X,
                                        op=mybir.AluOpType.add)
                # rstd = 1/sqrt(var/D + eps)
                rstd = Pw.tile([128, 1], dt.float32, tag="rstd")
                nc.scalar.activation(rstd[:, :], var[:, :],
                                     mybir.ActivationFunctionType.Sqrt,
                                     bias=eps_ln[:, :], scale=1.0 / D)
                nc.vector.reciprocal(rstd[:, :], rstd[:, :])
                # y = (xc * rstd) * gamma + beta   (reuse sq as output buf)
                nc.vector.scalar_tensor_tensor(
                    sq[:, :], xf[:, :], rstd[:, :], gam_bc[:, :],
                    op0=mybir.AluOpType.mult, op1=mybir.AluOpType.mult)
                nc.vector.tensor_add(sq[:, :], sq[:, :], bet_bc[:, :])
                nc.sync.dma_start(out_d[bass.ts(rc, 128), :], sq[:, :])

    nc.finalize()
    return nc


_PROGRAM = None


def _get_program():
    global _PROGRAM
    if _PROGRAM is None:
        _PROGRAM = build_program()
    return _PROGRAM


def _host_prep(h, Wq, Wkv, Wo, ln_gamma, ln_beta):
    h = np.asarray(h, F32)
    h_bm = np.ascontiguousarray(h.transpose(1, 0, 2).reshape(ROWS, D))
    hT = np.ascontiguousarray(h_bm.T).astype(BF16)
    Wq_h = np.asarray(Wq, F32).reshape(NH, DH, D)
    Wk_h = np.asarray(Wkv, F32)[:NH * DH].reshape(NH, DH, D)
    Wv_h = np.asarray(Wkv, F32)[NH * DH:].reshape(NH, DH, D)
    WoT = np.ascontiguousarray(np.asarray(Wo, F32).T).astype(BF16)
    mask4 = np.tile(np.triu(np.ones((128, 128), F32)), (1, 4)).astype(BF16)
    ident = np.eye(128, dtype=F32).astype(BF16)
    gamma = np.asarray(ln_gamma, F32).reshape(1, D)
    beta = np.asarray(ln_beta, F32).reshape(1, D)

    in_maps = []
    for core in range(N_CORES):
        hh = [HPC * core + i for i in range(HPC)]
        W_all = np.concatenate([
            np.concatenate([Wq_h[j] * S_FOLD for j in hh]),
            np.concatenate([Wk_h[j] for j in hh]),
            np.concatenate([Wv_h[j] for j in hh]),
        ])
        in_maps.append({
            "hT": hT,
            "wallT": np.ascontiguousarray(W_all.T).astype(BF16),
            "woT": WoT,
            "h_slice": np.ascontiguousarray(h_bm[core * RPC:(core + 1) * RPC]),
            "mask4": mask4,
            "ident": ident,
            "gamma": gamma,
            "beta": beta,
        })
    return in_maps


def run(inputs, trace=False):
    """Run on hardware; returns (output [SEQ,BATCH,D] f32, BassKernelResults)."""
    _install_profshim()
    nc = _get_program()
    in_maps = _host_prep(inputs["h"], inputs["Wq"], inputs["Wkv"], inputs["Wo"],
                         inputs["ln_gamma"], inputs["ln_beta"])
    res = run_bass_kernel_spmd(nc, in_maps, core_ids=list(range(N_CORES)),
                               trace=trace)
    out_bm = np.concatenate([res.results[c]["out"] for c in range(N_CORES)], axis=0)
    out = out_bm.reshape(BATCH, SEQ, D).transpose(1, 0, 2).astype(F32)
    return np.ascontiguousarray(out), res


def kernel(**inputs):
    out, _ = run(inputs, trace=False)
    return out


# revision 9
# speedup vs baseline: 1.1373x; 1.0797x over previous
"""Trainium2 Bass kernel for nn_MemTransformerLM (DPFP linear-attention block).

Full inputs in, full output out. Internally: head-shards across 8 NeuronCores
(2 heads/core), runs causal linear attention as a chunked prefix-sum (the
reference's sum-normalized kernelized attention factorizes: no SxS score
materialization), AllToAll re-shards heads->rows for the output projection,
and each core LayerNorms its row slice. Host concatenates the 8 row slices.

v2 structure (vs v1):
- per-head attention buffers so head-0's AllToAll stores don't falsely
  depend on head-1's copies (partition aliasing in dep tracking)
- attention emits u^T [d, i] directly (intra term va^T @ probT, state term
  KV^T @ qf) so no output transpose / scalar copy; normalization uses a
  K=1 PE broadcast of the reciprocal denominator row
- o-projection split per head: head-0's partial (x = hs + attn0 @ Wo0)
  runs while head-1's AllToAll is in flight, keeping the PE warm through
  the collective and leaving only head-1's partial + LayerNorm on the tail
- o-projection stacks rank pairs to use all 128 PE rows (K=128 not 64)
- phase-1/2 buffers freed before attention so phase-4 weights stay resident
- DPFP products split across Vector and GpSimd(Pool) engines
"""
import os
import sys
import types
from contextlib import ExitStack

for _p in ("/opt/trn_rl_repo",):
    if _p not in sys.path:
        sys.path.insert(0, _p)

import numpy as np
import ml_dtypes

import concourse.bass as bass
import concourse.mybir as mybir
import concourse.tile as tile
from concourse import bacc
from concourse.bass_utils import run_bass_kernel_spmd

BF16 = ml_dtypes.bfloat16
F32 = np.float32

SEQ, BATCH, D = 1536, 2, 1024
NH, DH, NR = 16, 64, 3
SCALE = 1.0 / float(np.sqrt(DH))
S_FOLD = float(np.sqrt(SCALE))           # folded into Wq rows (squared by DPFP products)
EPS_D, EPS_LN = 1e-5, 1e-5
N_CORES = 8
HPC = NH // N_CORES                      # heads per core (2)
ROWS = SEQ * BATCH                       # 3072 batch-major rows
RPC = ROWS // N_CORES                    # 384 output rows per core
NCHUNK = ROWS // 128                     # 24 chunks of 128 rows
NCB = NCHUNK // BATCH                    # 12 chunks per batch
FEAT = 2 * DH * NR                       # 384 DPFP features
NKD = D // 128                           # 8 contraction chunks over d_model
PW = 3 * HPC * DH                        # 384 projection width (q|k|v)

dt = mybir.dt

# chunk storage position: pos = cl*2 + b  (global chunk c = b*NCB + cl)
POS_OF_C = [(c % NCB) * 2 + (c // NCB) for c in range(NCHUNK)]
C_OF_POS = [0] * NCHUNK
for _c, _p in enumerate(POS_OF_C):
    C_OF_POS[_p] = _c


def _install_profshim():
    """Enable NTFF profiling under axon when antenv.axon_hooks is missing."""
    try:
        import antenv
    except ImportError:
        return
    if "antenv.axon_hooks" in sys.modules:
        return
    mod = types.ModuleType("antenv.axon_hooks")
    mod._hook = None
    mod.set_axon_ntff_profile_hook = lambda h: setattr(mod, "_hook", h)
    mod.get_axon_ntff_profile_hook = lambda: mod._hook
    sys.modules["antenv.axon_hooks"] = mod
    antenv.axon_hooks = mod
    try:
        from trn_agent_boot.trn_boot import _ntff_profile_via_ctypes
        mod.set_axon_ntff_profile_hook(
            _ntff_profile_via_ctypes("/opt/axon/libaxon_pjrt.so"))
    except Exception:
        pass


def build_program():
    nc = bacc.Bacc("TRN2", target_bir_lowering=False, debug=False,
                   num_devices=N_CORES)

    # ---- kernel I/O (per-core values supplied via in_maps) ----
    hT_d = nc.declare_dram_parameter("hT", [D, ROWS], dt.bfloat16, isOutput=False)
    wall_d = nc.declare_dram_parameter("wallT", [D, PW], dt.bfloat16, isOutput=False)
    woT_d = nc.declare_dram_parameter("woT", [D, D], dt.bfloat16, isOutput=False)
    hs_d = nc.declare_dram_parameter("h_slice", [RPC, D], dt.float32, isOutput=False)
    mask_d = nc.declare_dram_parameter("mask4", [128, 512], dt.bfloat16, isOutput=False)
    ident_d = nc.declare_dram_parameter("ident", [128, 128], dt.bfloat16, isOutput=False)
    gam_d = nc.declare_dram_parameter("gamma", [1, D], dt.float32, isOutput=False)
    bet_d = nc.declare_dram_parameter("beta", [1, D], dt.float32, isOutput=False)
    out_d = nc.declare_dram_parameter("out", [RPC, D], dt.float32, isOutput=True)

    # internal DRAM bounce buffers: one AllToAll per head
    a2a_in = [nc.dram_tensor(f"a2a_in{h}", [N_CORES, DH, RPC], dt.bfloat16)
              for h in range(HPC)]
    a2a_out = [nc.dram_tensor(f"a2a_out{h}", [N_CORES, DH, RPC], dt.bfloat16)
               for h in range(HPC)]

    with tile.TileContext(nc) as tc:
        with (
            tc.tile_pool(name="const", bufs=1) as Pc,
            tc.tile_pool(name="big", bufs=1) as Pb,
            tc.tile_pool(name="work", bufs=2) as Pw,
            tc.tile_pool(name="ps2", bufs=2, space="PSUM") as Pp,
            tc.tile_pool(name="ps_acc", bufs=1, space="PSUM") as Pacc,
            ExitStack() as _stack,
        ):
            _inner = ExitStack()
            Pi = _inner.enter_context(tc.tile_pool(name="inner", bufs=1))

            # ---------- constants ----------
            mask4 = Pc.tile([128, 512], dt.bfloat16, tag="mask4")
            ident = Pc.tile([128, 128], dt.bfloat16, tag="ident")
            gam = Pc.tile([1, D], dt.float32, tag="gam")
            bet = Pc.tile([1, D], dt.float32, tag="bet")
            ones1 = Pc.tile([1, 128], dt.float32, tag="ones1")
            eps_ln = Pc.tile([128, 1], dt.float32, tag="eps_ln")
            # ones row at partition 64 for the denominator broadcast matmul
            ones65 = Pc.tile([65, 64], dt.bfloat16, tag="ones65")
            eps65 = Pc.tile([65, 1], dt.float32, tag="eps65")
            nc.vector.memset(eps_ln[:, :], EPS_LN)
            nc.vector.memset(ones65[64:65, :], 1.0)
            nc.vector.memset(eps65[:, :], EPS_D)
            nc.sync.dma_start(mask4[:, :], mask_d[:, :])
            nc.sync.dma_start(ident[:, :], ident_d[:, :])
            nc.sync.dma_start(gam[:, :], gam_d[:, :])
            nc.sync.dma_start(bet[:, :], bet_d[:, :])
            nc.vector.memset(ones1[:, :], 1.0)
            # broadcast gamma/beta across partitions via PE (K=1 matmul)
            gam_bc = Pc.tile([128, D], dt.bfloat16, tag="gam_bc")
            bet_bc = Pc.tile([128, D], dt.bfloat16, tag="bet_bc")
            for half in range(2):
                gb_ps = Pp.tile([128, 512], dt.float32, tag="g_ps", bufs=4)
                nc.tensor.matmul(gb_ps[:, :], ones1[:1, :], gam[:1, bass.ts(half, 512)],
                                 start=True, stop=True)
                nc.scalar.copy(gam_bc[:, bass.ts(half, 512)], gb_ps[:, :])
                gb_ps2 = Pp.tile([128, 512], dt.float32, tag="g_ps", bufs=4)
                nc.tensor.matmul(gb_ps2[:, :], ones1[:1, :], bet[:1, bass.ts(half, 512)],
                                 start=True, stop=True)
                nc.scalar.copy(bet_bc[:, bass.ts(half, 512)], gb_ps2[:, :])

            # ---------- persistent big buffers (position-indexed columns) ----------
            # f2_all[p, pos*512 + ht*128 + j]: relu features, ht in (q0,q1,k0,k1)
            f2_all = Pi.tile([128, NCHUNK * 512], dt.bfloat16, tag="f2")
            # va_all[p, pos*130 + h*65 + d]: v augmented with ones column
            va_all = Pb.tile([128, NCHUNK * 130], dt.bfloat16, tag="va")
            # prodT[p, pos*384 + feat] per head-tensor (q0,q1 -> qfT; k0,k1 -> kfT)
            qfT = [Pb.tile([128, NCHUNK * FEAT], dt.bfloat16, tag=f"qfT{i}", name=f"qfT{i}")
                   for i in range(HPC)]
            kfT = [Pb.tile([128, NCHUNK * FEAT], dt.bfloat16, tag=f"kfT{i}", name=f"kfT{i}")
                   for i in range(HPC)]
            # attention output per head, [d, row] layout feeding the A2As
            attn_h = [Pb.tile([64, ROWS], dt.bfloat16, tag=f"attn{h}", name=f"attn{h}")
                      for h in range(HPC)]

            # ones columns of va (exact 1.0)
            va4 = va_all[:, :].rearrange("p (c h d) -> p c h d", h=2, d=65)
            nc.vector.memset(va4[:, :, :, 64:65], 1.0)

            # ---------- phase 1: projections + relu (position order) ----------
            w_sb = Pi.tile([128, NKD * PW], dt.bfloat16, tag="w_sb")
            for kd in range(NKD):
                nc.sync.dma_start(w_sb[:, bass.ts(kd, PW)], wall_d[bass.ts(kd, 128), :])
            # hT loaded as 8 big contiguous DMAs (one per 128-row d_model chunk)
            ht_sb = [Pi.tile([128, ROWS], dt.bfloat16, tag=f"ht{kd}", name=f"ht{kd}")
                     for kd in range(NKD)]
            for kd in range(NKD):
                nc.sync.dma_start(ht_sb[kd][:, :], hT_d[bass.ts(kd, 128), :])

            for pos in range(NCHUNK):
                c = C_OF_POS[pos]
                pps = Pp.tile([128, 512], dt.float32, tag="g_ps", bufs=4)
                for kd in range(NKD):
                    nc.tensor.matmul(pps[:, 0:PW], ht_sb[kd][:, bass.ts(c, 128)],
                                     w_sb[:, bass.ts(kd, PW)],
                                     start=(kd == 0), stop=(kd == NKD - 1))
                # relu(+x) on Scalar, relu(-x) on Vector -> f2 blocks [relu|relu-]
                f2c = f2_all[:, bass.ts(pos, 512)].rearrange("p (b s) -> p b s", b=4, s=128)
                pq = pps[:, 0:256].rearrange("p (b s) -> p b s", b=4, s=64)
                nc.scalar.activation(f2c[:, :, 0:64], pq[:, :, :],
                                     mybir.ActivationFunctionType.Relu)
                nc.scalar.activation(f2c[:, :, 64:128], pq[:, :, :],
                                     mybir.ActivationFunctionType.Relu, scale=-1.0)
                # v copy into augmented layout
                vac = va_all[:, bass.ts(pos, 130)].rearrange("p (h d) -> p h d", h=2, d=65)
                pv = pps[:, 256:384].rearrange("p (h d) -> p h d", h=2, d=64)
                nc.scalar.copy(vac[:, :, 0:64], pv[:, :, :])

            # ---------- phase 2: DPFP roll products, two position-groups ----------
            # q0/k0 (head 0) on Vector; q1/k1 (head 1) on GpSimd(Pool)
            for grp in range(2):
                sl = slice(grp * 12, (grp + 1) * 12)
                f2r = f2_all[:, :].rearrange("p (c b j) -> p c b j", b=4, j=128)[:, sl]
                for ht in (0, 2, 1, 3):              # q0 k0 first (head 0 unblocks)
                    eng = nc.vector
                    dst = (qfT if ht < 2 else kfT)[ht % 2]
                    dstr = dst[:, :].rearrange("p (c t j) -> p c t j", t=NR, j=128)[:, sl]
                    for t in range(1, NR + 1):
                        eng.tensor_mul(dstr[:, :, t - 1, t:128],
                                       f2r[:, :, ht, t:128],
                                       f2r[:, :, ht, 0:128 - t])
                        eng.tensor_mul(dstr[:, :, t - 1, 0:t],
                                       f2r[:, :, ht, 0:t],
                                       f2r[:, :, ht, 128 - t:128])

            _inner.close()   # frees hT / f2 / w_sb SBUF space before attention

            # ---------- phase-4 persistent buffers + weight prefetch ----------
            Po = _stack.enter_context(tc.tile_pool(name="post", bufs=1))
            # wo2[h]: rank-pair-stacked Wo rows: partitions 0-63 <- rank 2q,
            # 64-127 <- rank 2q+1 (full-K o-projection matmuls)
            wo2 = [Po.tile([128, 4 * D], dt.bfloat16, tag=f"wo{h}", name=f"wo{h}")
                   for h in range(HPC)]
            for h in range(HPC):
                for q in range(4):
                    nc.sync.dma_start(
                        wo2[h][0:64, bass.ts(q, D)],
                        woT_d[(2 * q) * 128 + h * 64:(2 * q) * 128 + (h + 1) * 64, :])
                    nc.sync.dma_start(
                        wo2[h][64:128, bass.ts(q, D)],
                        woT_d[(2 * q + 1) * 128 + h * 64:(2 * q + 1) * 128 + (h + 1) * 64, :])
            hs_sb = [Po.tile([128, D], dt.float32, tag=f"hs{rc}", name=f"hs{rc}")
                     for rc in range(3)]
            for rc in range(3):
                nc.sync.dma_start(hs_sb[rc][:, :], hs_d[bass.ts(rc, 128), :])
            x_sb = [Po.tile([128, D], dt.float32, tag=f"x{rc}", name=f"x{rc}")
                    for rc in range(3)]
            # asl2[h]: rank-pair-stacked gathered attention (partitions as wo2)
            asl2 = [Po.tile([128, 4 * RPC], dt.bfloat16, tag=f"asl{h}", name=f"asl{h}")
                    for h in range(HPC)]

            # ---------- phase 3: attention, head-outer; A2A per head ----------
            def emit_norm(h, cl, uT):
                """attn[d,i] = uT[d,i]/(uT[64,i]+eps): denom row +eps (ACT),
                K=1 PE broadcast into the dead score bank, fast recip (DVE),
                per-batch mul writing contiguous attn_h columns. Emitted one
                iteration late so the PE queue never stalls on the ACT hop."""
                dsb = Pw.tile([65, 256], dt.bfloat16, tag="dsb", name="dsb")
                nc.scalar.activation(dsb[64:65, :], uT[64:65, :],
                                     mybir.ActivationFunctionType.Copy,
                                     bias=EPS_D)
                bc_full = Pp.tile([64, 512], dt.float32, tag="sc_ps", bufs=1,
                                  name="bc")
                bc = bc_full[:, 0:256]
                nc.tensor.matmul(bc[:, :], ones65[64:65, :], dsb[64:65, :],
                                 start=True, stop=True)
                rec = Pw.tile([64, 256], dt.float32, tag="rec", name="rec")
                nc.vector.reciprocal_approx_fast(rec[:, :], bc[:, :])
                for b in range(BATCH):
                    nc.vector.tensor_mul(
                        attn_h[h][:, b * SEQ + cl * 128:b * SEQ + (cl + 1) * 128],
                        uT[0:64, bass.ts(b, 128)],
                        rec[:, bass.ts(b, 128)])

            for h in range(HPC):
                kv_acc = Pacc.tile([128, 390], dt.float32, tag="kvp", name=f"kvp{h}")
                kv_sb = None
                pend = None
                for cl in range(NCB):
                    # transposed feature chunks [feat, i], both batches packed
                    # into one PSUM bank / one SBUF tile per q/k
                    tq2 = Pw.tile([128, 768], dt.bfloat16, tag="qf_c", bufs=2)
                    tk2 = Pw.tile([128, 768], dt.bfloat16, tag="kf_c", bufs=2)
                    psq = Pp.tile([128, 768], dt.bfloat16, tag="g_ps", bufs=4)
                    psk = Pp.tile([128, 768], dt.bfloat16, tag="g_ps", bufs=4)
                    for b in range(BATCH):
                        pos = cl * 2 + b
                        for t in range(NR):
                            nc.tensor.transpose(
                                psq[:, b * 384 + t * 128:b * 384 + (t + 1) * 128],
                                qfT[h][:, pos * FEAT + t * 128:pos * FEAT + (t + 1) * 128],
                                ident[:, :])
                            nc.tensor.transpose(
                                psk[:, b * 384 + t * 128:b * 384 + (t + 1) * 128],
                                kfT[h][:, pos * FEAT + t * 128:pos * FEAT + (t + 1) * 128],
                                ident[:, :])
                    nc.scalar.copy(tq2[:, :], psq[:, :])
                    nc.vector.tensor_copy(tk2[:, :], psk[:, :])

                    # scoreT[j, i] both batches in one PSUM bank
                    sc_ps = Pp.tile([128, 512], dt.float32, tag="sc_ps", bufs=1)
                    for b in range(BATCH):
                        for t in range(NR):
                            nc.tensor.matmul(sc_ps[:, bass.ts(b, 128)],
                                             tk2[:, b * 384 + t * 128:b * 384 + (t + 1) * 128],
                                             tq2[:, b * 384 + t * 128:b * 384 + (t + 1) * 128],
                                             start=(t == 0), stop=(t == NR - 1))
                    probT = Pw.tile([128, 256], dt.bfloat16, tag="probT")
                    nc.vector.tensor_mul(probT[:, :], sc_ps[:, 0:256], mask4[:, 0:256])

                    # uT[d, i]: d 0-63 = unnorm attn, d 64 = denom; intra + state
                    uT_full = Pp.tile([65, 512], dt.float32, tag="uT", bufs=2)
                    uT = uT_full[:, 0:256]
                    for b in range(BATCH):
                        pos = cl * 2 + b
                        va_c = va_all[:, pos * 130 + h * 65:pos * 130 + (h + 1) * 65]
                        nc.tensor.matmul(uT[:, bass.ts(b, 128)], va_c,
                                         probT[:, bass.ts(b, 128)],
                                         start=True, stop=(cl == 0))
                        if cl > 0:
                            for t in range(NR):
                                nc.tensor.matmul(uT[:, bass.ts(b, 128)],
                                                 kv_sb[b][:, bass.ts(t, 65)],
                                                 tq2[:, b * 384 + t * 128:b * 384 + (t + 1) * 128],
                                                 start=False, stop=(t == NR - 1))

                    # KV state update: KV += kfT_c.T @ va_c  (PSUM accumulator)
                    kv_pk = Pw.tile([128, 390], dt.bfloat16, tag="kv_pk", bufs=2)
                    kv_sb_next = [kv_pk[:, bass.ts(b, 195)] for b in range(BATCH)]
                    for b in range(BATCH):
                        pos = cl * 2 + b
                        va_c = va_all[:, pos * 130 + h * 65:pos * 130 + (h + 1) * 65]
                        for t in range(NR):
                            # start only on the very first touch of this bank
                            # (start marks the whole 2KB zero region pending)
                            nc.tensor.matmul(
                                kv_acc[:, b * 195 + t * 65:b * 195 + (t + 1) * 65],
                                kfT[h][:, pos * FEAT + t * 128:pos * FEAT + (t + 1) * 128],
                                va_c,
                                start=(cl == 0 and b == 0 and t == 0),
                                stop=(cl == NCB - 1),
                                skip_group_check=True)
                    if cl < NCB - 1:
                        nc.scalar.copy(kv_pk[:, :], kv_acc[:, :])
                    kv_sb = kv_sb_next

                    if pend is not None:
                        emit_norm(h, *pend)
                    pend = (cl, uT)
                emit_norm(h, *pend)

                # ---- AllToAll for this head ----
                for r in range(N_CORES):
                    nc.sync.dma_start(a2a_in[h][r, :, :],
                                      attn_h[h][:, bass.ts(r, RPC)])
                nc.gpsimd.collective_compute(
                    "AllToAll", mybir.AluOpType.bypass,
                    replica_groups=[list(range(N_CORES))],
                    ins=[a2a_in[h].ap().opt()], outs=[a2a_out[h].ap().opt()])

            # ---------- phase 4a: head-0 partial o-projection (hides A2A#1) ----
            for h in range(HPC):
                if h == 1:
                    # hold the PE clock warm across the exposed A2A#1 window
                    # (idle > ~3.4us re-throttles HAM to half clock)
                    for _w in range(24):
                        warm = Pp.tile([128, 512], dt.float32, tag="g_ps",
                                       bufs=4, name=f"warm{_w}")
                        nc.tensor.matmul(warm[:, :], ident[:, :],
                                         gam_bc[:, 0:512], start=True, stop=True)
                for q in range(4):
                    nc.sync.dma_start(asl2[h][0:64, bass.ts(q, RPC)],
                                      a2a_out[h][2 * q, :, :])
                    nc.sync.dma_start(asl2[h][64:128, bass.ts(q, RPC)],
                                      a2a_out[h][2 * q + 1, :, :])
                for rc in range(3):
                    for n in range(2):
                        ops = Pp.tile([128, 512], dt.float32, tag="g_ps", bufs=4)
                        for q in range(4):
                            nc.tensor.matmul(
                                ops[:, :],
                                asl2[h][:, q * RPC + rc * 128:q * RPC + (rc + 1) * 128],
                                wo2[h][:, q * D + n * 512:q * D + (n + 1) * 512],
                                start=(q == 0), stop=(q == 3))
                        if h == 0:
                            # x = attn0-part + h_slice
                            nc.vector.scalar_tensor_tensor(
                                x_sb[rc][:, bass.ts(n, 512)], ops[:, :], 0.0,
                                hs_sb[rc][:, bass.ts(n, 512)],
                                op0=mybir.AluOpType.add, op1=mybir.AluOpType.add)
                        else:
                            # xf = attn1-part + x ; accumulate row-sum for mean
                            xf = _XF[rc]
                            nc.vector.scalar_tensor_tensor(
                                xf[:, bass.ts(n, 512)], ops[:, :], 0.0,
                                x_sb[rc][:, bass.ts(n, 512)],
                                op0=mybir.AluOpType.add, op1=mybir.AluOpType.add,
                                accum_out=_S2[rc][:, n:n + 1])
                if h == 0:
                    # allocate LN scratch between the two passes
                    _XF = [Po.tile([128, D], dt.float32, tag=f"xf{rc}", name=f"xf{rc}")
                           for rc in range(3)]
                    _S2 = [Pw.tile([128, 2], dt.float32, tag=f"s2_{rc}",
                                   name=f"s2_{rc}") for rc in range(3)]

            # ---------- phase 4b: layernorm + store ----------
            for rc in range(3):
                xf = _XF[rc]
                s2 = _S2[rc]
                mean = Pw.tile([128, 1], dt.float32, tag="mean")
                nc.vector.tensor_reduce(mean[:, :], s2[:, :],
                                        axis=mybir.AxisListType.X,
                                        op=mybir.AluOpType.add)
                nc.vector.tensor_scalar_mul(mean[:, :], mean[:, :], 1.0 / D)
                negm = Pw.tile([128, 1], dt.float32, tag="negm")
                nc.vector.tensor_scalar_mul(negm[:, :], mean[:, :], -1.0)
                # var = sum((x - mean)^2) via ACT Square-accumulate (x stays
                # uncentered; the squared tensor itself is a dead write)
                var = Pw.tile([128, 1], dt.float32, tag="var")
                sq = Po.tile([128, D], dt.float32, tag="sq", bufs=2)
                nc.scalar.activation(sq[:, :], xf[:, :],
                                     mybir.ActivationFunctionType.Square,
                                     bias=negm[:, :], accum_out=var[:, :])
                # rstd = 1/sqrt(var/D + eps)
                rstd = Pw.tile([128, 1], dt.float32, tag="rstd")
                nc.scalar.activation(rstd[:, :], var[:, :],
                                     mybir.ActivationFunctionType.Sqrt,
                                     bias=eps_ln[:, :], scale=1.0 / D)
                nc.vector.reciprocal(rstd[:, :], rstd[:, :])
                # xc = (x - mean) * rstd ; y = xc * gamma (DVE) + beta (Pool)
                xc = Po.tile([128, D], dt.float32, tag="xc", bufs=2)
                nc.vector.tensor_scalar(xc[:, :], xf[:, :], mean[:, :],
                                        rstd[:, :],
                                        op0=mybir.AluOpType.subtract,
                                        op1=mybir.AluOpType.mult)
                nc.vector.tensor_mul(sq[:, :], xc[:, :], gam_bc[:, :])
                nc.gpsimd.tensor_add(sq[:, :], sq[:, :], bet_bc[:, :])
                nc.sync.dma_start(out_d[bass.ts(rc, 128), :], sq[:, :])

    nc.finalize()
    return nc


_PROGRAM = None


def _get_program():
    global _PROGRAM
    if _PROGRAM is None:
        _PROGRAM = build_program()
    return _PROGRAM


def _host_prep(h, Wq, Wkv, Wo, ln_gamma, ln_beta):
    h = np.asarray(h, F32)
    h_bm = np.ascontiguousarray(h.transpose(1, 0, 2).reshape(ROWS, D))
    hT = np.ascontiguousarray(h_bm.T).astype(BF16)
    Wq_h = np.asarray(Wq, F32).reshape(NH, DH, D)
    Wk_h = np.asarray(Wkv, F32)[:NH * DH].reshape(NH, DH, D)
    Wv_h = np.asarray(Wkv, F32)[NH * DH:].reshape(NH, DH, D)
    WoT = np.ascontiguousarray(np.asarray(Wo, F32).T).astype(BF16)
    mask4 = np.tile(np.triu(np.ones((128, 128), F32)), (1, 4)).astype(BF16)
    ident = np.eye(128, dtype=F32).astype(BF16)
    gamma = np.asarray(ln_gamma, F32).reshape(1, D)
    beta = np.asarray(ln_beta, F32).reshape(1, D)

    in_maps = []
    for core in range(N_CORES):
        hh = [HPC * core + i for i in range(HPC)]
        W_all = np.concatenate([
            np.concatenate([Wq_h[j] * S_FOLD for j in hh]),
            np.concatenate([Wk_h[j] for j in hh]),
            np.concatenate([Wv_h[j] for j in hh]),
        ])
        in_maps.append({
            "hT": hT,
            "wallT": np.ascontiguousarray(W_all.T).astype(BF16),
            "woT": WoT,
            "h_slice": np.ascontiguousarray(h_bm[core * RPC:(core + 1) * RPC]),
            "mask4": mask4,
            "ident": ident,
            "gamma": gamma,
            "beta": beta,
        })
    return in_maps


def run(inputs, trace=False):
    """Run on hardware; returns (output [SEQ,BATCH,D] f32, BassKernelResults)."""
    _install_profshim()
    nc = _get_program()
    in_maps = _host_prep(inputs["h"], inputs["Wq"], inputs["Wkv"], inputs["Wo"],
                         inputs["ln_gamma"], inputs["ln_beta"])
    res = run_bass_kernel_spmd(nc, in_maps, core_ids=list(range(N_CORES)),
                               trace=trace)
    out_bm = np.concatenate([res.results[c]["out"] for c in range(N_CORES)], axis=0)
    out = out_bm.reshape(BATCH, SEQ, D).transpose(1, 0, 2).astype(F32)
    return np.ascontiguousarray(out), res


def kernel(**inputs):
    out, _ = run(inputs, trace=False)
    return out
